# revision 1
# baseline (speedup 1.0000x reference)
"""Conv2D 3x3 (stride 1, pad 1) NCHW on 8 TRN2 NeuronCores.

x: (32, 128, 56, 56) f32, weight: (256, 128, 3, 3) OIHW, bias: (256,)
out: (32, 256, 56, 56) f32.

Strategy: data-parallel over batch (4 images per core, weight/bias
replicated). The input is zero-padded to 58x58 on the host, so each padded
image lives in SBUF with C_in=128 on partitions and needs no on-device
border handling. The 3x3 conv is 9 shifted [128x128] @ [128x448] matmuls
accumulated in PSUM (output tile = 8 rows x 56 cols per co-tile), using
float32r operands (full PE rate, ~1.5e-4 rel err). Bias is added on the
vector engine while evacuating PSUM -> SBUF, then DMA to HBM.
"""

import numpy as np

import concourse.tile as tile
from concourse import bacc, mybir
from concourse.bass_utils import run_bass_kernel_spmd

N_CORES = 8
N_BATCH = 32
N_PER_CORE = N_BATCH // N_CORES  # 4
C_IN, C_OUT, H, W = 128, 256, 56, 56
HP, WP = H + 2, W + 2  # 58 (zero-padded on host)
ROWS = 8  # output rows per PSUM tile
N_RTILES = H // ROWS  # 7
NFREE = ROWS * W  # 448 <= 512 (one PSUM bank; f32r full rate needs >= 256)
N_CT = C_OUT // 128  # 2 co-tiles


def build_nc(n_imgs=N_PER_CORE):
    f32 = mybir.dt.float32
    f32r = mybir.dt.float32r
    nc = bacc.Bacc("TRN2", target_bir_lowering=False, debug=False)
    x = nc.dram_tensor("x", [n_imgs, C_IN, HP, WP], f32r, kind="ExternalInput")
    w = nc.dram_tensor("w", [C_IN, 9 * C_OUT], f32r, kind="ExternalInput")
    b = nc.dram_tensor("b", [C_IN, N_CT], f32, kind="ExternalInput")
    out = nc.dram_tensor("out", [n_imgs, C_OUT, H * W], f32, kind="ExternalOutput")

    with tile.TileContext(nc) as tc:
        with tc.tile_pool(name="wpool", bufs=1) as wpool, \
             tc.tile_pool(name="xpool", bufs=2) as xpool, \
             tc.tile_pool(name="opool", bufs=8) as opool, \
             tc.tile_pool(name="pspool", bufs=4, space="PSUM") as pspool:
            # Startup is posting-bound: descriptor generation costs ~0.6us
            # per DMA per sequencer, so interleave posts across the sync and
            # (idle-at-startup) scalar sequencers. Order: the first two
            # image-0 chunks (the first row-tile's inputs), then the nine
            # weight taps, then the rest of image 0.
            w_sb = wpool.tile([C_IN, 9 * C_OUT], f32r)
            b_sb = wpool.tile([C_IN, N_CT], f32)
            xp0 = xpool.tile([C_IN, HP, WP], f32r, tag="xp", name="xp")
            x_chunks = [(a, min(a + ROWS, HP)) for a in range(0, HP, ROWS)]
            nc.sync.dma_start(xp0[:, x_chunks[0][0]:x_chunks[0][1], :],
                              x[0, :, x_chunks[0][0]:x_chunks[0][1], :])
            nc.scalar.dma_start(xp0[:, x_chunks[1][0]:x_chunks[1][1], :],
                                x[0, :, x_chunks[1][0]:x_chunks[1][1], :])
            for tap in range(9):
                sl = slice(tap * C_OUT, (tap + 1) * C_OUT)
                eng = nc.sync if tap % 2 == 0 else nc.scalar
                eng.dma_start(w_sb[:, sl], w[:, sl])
            nc.sync.dma_start(b_sb[:], b[:])
            for ci, (a, e) in enumerate(x_chunks[2:]):
                eng = nc.scalar if ci % 2 == 0 else nc.sync
                eng.dma_start(xp0[:, a:e, :], x[0, :, a:e, :])

            for n in range(n_imgs):
                if n == 0:
                    xp = xp0
                else:
                    xp = xpool.tile([C_IN, HP, WP], f32r, tag="xp", name="xp")
                    # later images prefetch under compute; HWDGE via sync
                    # (SWDGE descriptor traffic slows the PE's SBUF stream)
                    for a, e in x_chunks:
                        nc.sync.dma_start(xp[:, a:e, :], x[n, :, a:e, :])
                for r in range(N_RTILES):
                    for ct in range(N_CT):
                        pt = pspool.tile([128, NFREE], f32, tag="pt")
                        for tap in range(9):
                            kh, kw = tap // 3, tap % 3
                            c0 = tap * C_OUT + ct * 128
                            nc.tensor.matmul(
                                pt[:],
                                w_sb[:, c0:c0 + 128],
                                xp[:, r * ROWS + kh:r * ROWS + kh + ROWS, kw:kw + W],
                                start=(tap == 0),
                                stop=(tap == 8),
                            )
                        ot = opool.tile([128, NFREE], f32, tag="ot")
                        nc.vector.tensor_scalar_add(ot[:], pt[:], b_sb[:, ct:ct + 1])
                        # outputs on sync in halves; the final tile in
                        # quarters so the post-compute drain tail is short
                        last = n == n_imgs - 1 and r == N_RTILES - 1
                        parts = 4 if last else 2
                        step = NFREE // parts
                        for hh in range(parts):
                            nc.sync.dma_start(
                                out[n, ct * 128:(ct + 1) * 128,
                                    r * NFREE + hh * step:r * NFREE + (hh + 1) * step],
                                ot[:, hh * step:(hh + 1) * step],
                            )
    nc.compile()
    return nc


def _host_prep(x, weight, bias):
    # zero-pad H and W by 1 on the host: border handling costs nothing here
    xp = np.pad(np.asarray(x, dtype=np.float32),
                ((0, 0), (0, 0), (1, 1), (1, 1)))
    xp = np.ascontiguousarray(xp)
    # weight OIHW -> [ci, (kh kw co)] so each lhsT tile is a contiguous slice
    w_host = np.ascontiguousarray(
        np.asarray(weight, dtype=np.float32).transpose(1, 2, 3, 0).reshape(C_IN, 9 * C_OUT)
    )
    # bias[co] -> [co % 128, co // 128]
    b_host = np.ascontiguousarray(
        np.asarray(bias, dtype=np.float32).reshape(N_CT, 128).T)
    return xp, w_host, b_host


def kernel(x, weight, bias, _trace=False):
    xp, w_host, b_host = _host_prep(x, weight, bias)
    nc = build_nc()
    in_maps = [
        {"x": xp[i * N_PER_CORE:(i + 1) * N_PER_CORE], "w": w_host, "b": b_host}
        for i in range(N_CORES)
    ]
    res = run_bass_kernel_spmd(nc, in_maps, core_ids=list(range(N_CORES)), trace=_trace)
    out = np.concatenate(
        [res.results[i]["out"].reshape(N_PER_CORE, C_OUT, H, W) for i in range(N_CORES)],
        axis=0,
    )
    if _trace:
        return out, res
    return out



# revision 2
# speedup vs baseline: 1.1175x; 1.1175x over previous
"""Conv2D 3x3 (stride 1, pad 1) NCHW on 8 TRN2 NeuronCores.

x: (32, 128, 56, 56) f32, weight: (256, 128, 3, 3) OIHW, bias: (256,)
out: (32, 256, 56, 56) f32.

Strategy: data-parallel over batch (4 images per core, weight/bias
replicated). The input is zero-padded to 58x58 and converted to bf16 on the
host (rel err ~3e-3 << 2e-2 gate; PSUM accumulates in f32). Each padded
image lives in SBUF with C_in=128 on partitions; the 3x3 conv is 9 shifted
[128x128] @ [128x448] bf16 matmuls accumulated in PSUM (output tile = 8
rows x 56 cols per co-tile). bf16 enables the compiler's Fast Weight Load,
so the per-matmul LDWEIGHTS (~96ns) hides behind the 187ns matmul stream.

DMA plan: the two HWDGE rings are independent FIFOs — inputs ride the
scalar ring (weights tap0 first, then the rest; image 0 in 3 chunks with
the first 10 rows leading so the first matmul can start ~1.5us in; later
images prefetch as 2 big chunks each), outputs ride the sync ring (one DMA
per [128co x 448pix] tile) so the post-compute drain tail is just the last
tile's bias-add + store.
"""

import numpy as np
import ml_dtypes

import concourse.tile as tile
from concourse import bacc, mybir
from concourse.bass_utils import run_bass_kernel_spmd

N_CORES = 8
N_BATCH = 32
N_PER_CORE = N_BATCH // N_CORES  # 4
C_IN, C_OUT, H, W = 128, 256, 56, 56
HP, WP = H + 2, W + 2  # 58 (zero-padded on host)
ROWS = 8  # output rows per PSUM tile
N_RTILES = H // ROWS  # 7
NFREE = ROWS * W  # 448 <= 512 (one PSUM bank of f32)
N_CT = C_OUT // 128  # 2 co-tiles


def build_nc(n_imgs=N_PER_CORE):
    f32 = mybir.dt.float32
    bf16 = mybir.dt.bfloat16
    nc = bacc.Bacc("TRN2", target_bir_lowering=False, debug=False)
    x = nc.dram_tensor("x", [n_imgs, C_IN, HP, WP], bf16, kind="ExternalInput")
    w = nc.dram_tensor("w", [C_IN, 9 * C_OUT], bf16, kind="ExternalInput")
    b = nc.dram_tensor("b", [C_IN, N_CT], f32, kind="ExternalInput")
    out = nc.dram_tensor("out", [n_imgs, C_OUT, H * W], f32, kind="ExternalOutput")

    with tile.TileContext(nc) as tc:
        with tc.tile_pool(name="wpool", bufs=1) as wpool, \
             tc.tile_pool(name="xpool", bufs=3) as xpool, \
             tc.tile_pool(name="opool", bufs=8) as opool, \
             tc.tile_pool(name="pspool", bufs=4, space="PSUM") as pspool:
            w_sb = wpool.tile([C_IN, 9 * C_OUT], bf16)
            b_sb = wpool.tile([C_IN, N_CT], f32)
            xp0 = xpool.tile([C_IN, HP, WP], bf16, tag="xp", name="xp")
            # Startup critical path: the first row-tile's matmuls need only
            # weight taps (kh,0..2) and input rows 0-9. Lead each ring with
            # exactly that, everything else follows.
            nc.sync.dma_start(xp0[:, 0:10, :], x[0, :, 0:10, :])
            nc.scalar.dma_start(w_sb[:, 0:3 * C_OUT], w[:, 0:3 * C_OUT])
            nc.sync.dma_start(xp0[:, 10:34, :], x[0, :, 10:34, :])
            nc.scalar.dma_start(w_sb[:, 3 * C_OUT:], w[:, 3 * C_OUT:])
            nc.scalar.dma_start(b_sb[:], b[:])
            nc.sync.dma_start(xp0[:, 34:58, :], x[0, :, 34:58, :])

            for n in range(n_imgs):
                if n == 0:
                    xp = xp0
                else:
                    xp = xpool.tile([C_IN, HP, WP], bf16, tag="xp", name="xp")
                    nc.scalar.dma_start(xp[:, 0:29, :], x[n, :, 0:29, :])
                    nc.scalar.dma_start(xp[:, 29:58, :], x[n, :, 29:58, :])
                for r in range(N_RTILES):
                    for ct in range(N_CT):
                        pt = pspool.tile([128, NFREE], f32, tag="pt")
                        for tap in range(9):
                            kh, kw = tap // 3, tap % 3
                            c0 = tap * C_OUT + ct * 128
                            nc.tensor.matmul(
                                pt[:],
                                w_sb[:, c0:c0 + 128],
                                xp[:, r * ROWS + kh:r * ROWS + kh + ROWS, kw:kw + W],
                                start=(tap == 0),
                                stop=(tap == 8),
                            )
                        last = n == n_imgs - 1 and r == N_RTILES - 1 and ct == N_CT - 1
                        parts = 2 if last else 1
                        step = NFREE // parts
                        for hh in range(parts):
                            ot = opool.tile([128, step], f32, tag="ot")
                            nc.vector.tensor_scalar_add(
                                ot[:], pt[:, hh * step:(hh + 1) * step],
                                b_sb[:, ct:ct + 1])
                            nc.sync.dma_start(
                                out[n, ct * 128:(ct + 1) * 128,
                                    r * NFREE + hh * step:r * NFREE + (hh + 1) * step],
                                ot[:],
                            )
    nc.compile()
    return nc


def _host_prep(x, weight, bias):
    # zero-pad H and W by 1 on the host, convert to bf16 (RTNE)
    xp = np.pad(np.asarray(x, dtype=np.float32),
                ((0, 0), (0, 0), (1, 1), (1, 1)))
    xp = np.ascontiguousarray(xp.astype(ml_dtypes.bfloat16))
    # weight OIHW -> [ci, (kh kw co)] so each lhsT tile is a contiguous slice
    w_host = np.ascontiguousarray(
        np.asarray(weight, dtype=np.float32)
        .transpose(1, 2, 3, 0).reshape(C_IN, 9 * C_OUT)
        .astype(ml_dtypes.bfloat16)
    )
    # bias[co] -> [co % 128, co // 128]
    b_host = np.ascontiguousarray(
        np.asarray(bias, dtype=np.float32).reshape(N_CT, 128).T)
    return xp, w_host, b_host


def kernel(x, weight, bias, _trace=False):
    xp, w_host, b_host = _host_prep(x, weight, bias)
    nc = build_nc()
    in_maps = [
        {"x": xp[i * N_PER_CORE:(i + 1) * N_PER_CORE], "w": w_host, "b": b_host}
        for i in range(N_CORES)
    ]
    res = run_bass_kernel_spmd(nc, in_maps, core_ids=list(range(N_CORES)), trace=_trace)
    out = np.concatenate(
        [res.results[i]["out"].reshape(N_PER_CORE, C_OUT, H, W) for i in range(N_CORES)],
        axis=0,
    )
    if _trace:
        return out, res
    return out
